# revision 3
# baseline (speedup 1.0000x reference)
"""Trainium2 Bass kernel for nn_ConcatLayer_37589553774933 (topk_masking).

Per-row computation on [N, 9] f32 (N = 8388608): three groups of 3
(up/none/down); per group a strict-argmax code in {-1,0,1}; a scalar
decision chain (calc/sign/idx); masking; probe-argmax group selection;
output [N, 3]. Rows are split evenly across 8 NeuronCores (SPMD).

Engine balance per tile (C=512 rows/partition):
- Pool (add/sub/mult only): DA = v0-X, DB = v2-Y, KD = m-sgn,
  PR = PRraw*keep, OT = OTraw*ksel
- DVE: X/Y pairwise maxes, strict-compare bits via 2x tensor_scalar
  (is_gt vs 0 on Pool-computed differences), bf16 code/calc chain,
  copy_predicated selects, probe argmax compares
- ACT: sign(calc), select-default copies (PRraw/OTraw/KSEL)
All comparisons are exact f32; bf16 only holds small exact ints.
"""


import os

import numpy as np

import concourse.bass as bass
import concourse.mybir as mybir
from concourse.tile import TileContext

F32 = mybir.dt.float32
BF16 = mybir.dt.bfloat16
U8 = mybir.dt.uint8
OP = mybir.AluOpType

N_TOTAL = 8388608
N_CORES = 8
R_CORE = N_TOTAL // N_CORES
P = 128


def _copy_pred(eng, out, mask, data):
    return eng.add_instruction(
        mybir.InstCopyPredicated(
            name=f"I-{eng.bass.next_id()}",
            ins=[eng.lower_ap(mask, opt=False), eng.lower_ap(data, opt=False)],
            outs=[eng.lower_ap(out, opt=False)],
        )
    )


def split_multi_waits(nc, max_waits: int = 1):
    n_split = 0
    for f in nc.m.functions:
        for b in f.blocks:
            new_insts = []
            for ins in b.instructions:
                si = getattr(ins, "sync_info", None)
                if si and si.on_wait and len(si.on_wait) > max_waits:
                    waits = list(si.on_wait)
                    head, tail = waits[:-max_waits], waits[-max_waits:]
                    for k in range(0, len(head), max_waits):
                        chunk = head[k : k + max_waits]
                        nop = mybir.InstNoOp(
                            name=f"{ins.name}_waitsplit{k}",
                            ins=[],
                            outs=[],
                            sync_info=mybir.SyncInfo(
                                on_wait=list(chunk), on_update=[]
                            ),
                        )
                        nop.engine = ins.engine
                        new_insts.append(nop)
                    si.on_wait = tail
                    n_split += 1
                new_insts.append(ins)
            b.instructions = new_insts
    return n_split


def build_nc(R: int, C: int, bufs: int = 2):
    T = R // (P * C)
    assert R == T * P * C, (R, C)
    nc = bass.Bass("TRN2", debug=False)
    x = nc.declare_dram_parameter("x", [R, 9], F32, isOutput=False)
    o = nc.declare_dram_parameter("o", [R, 3], F32, isOutput=True)
    xt = x[:].rearrange("(t p c) j -> t p (c j)", t=T, p=P, c=C)
    ot = o[:].rearrange("(t p c) j -> t p (c j)", t=T, p=P, c=C)

    dve = nc.vector
    gp = nc.gpsimd
    act = nc.scalar

    with TileContext(nc) as tc:
        with tc.tile_pool(name="pool", bufs=bufs) as pool:
            for t in range(T):
                tin = pool.tile([P, 9 * C], F32, name="tin")
                nc.sync.dma_start(tin[:], xt[t])
                tv = tin[:].rearrange("p (c g j) -> p c g j", c=C, g=3, j=3)
                V = [tv[:, :, :, j] for j in range(3)]   # [P,C,3g] stride-3
                G = [tv[:, :, g, :] for g in range(3)]   # [P,C,3j] contig j

                # --- group codes ---------------------------------------
                X = pool.tile([P, 3 * C], F32, name="X")
                Xv = X[:].rearrange("p (c g) -> p c g", g=3)
                dve.tensor_tensor(Xv, V[1], V[2], op=OP.max)
                Y = pool.tile([P, 3 * C], F32, name="Y")
                Yv = Y[:].rearrange("p (c g) -> p c g", g=3)
                dve.tensor_tensor(Yv, V[0], V[1], op=OP.max)

                DA = pool.tile([P, 3 * C], F32, name="DA")
                DAv = DA[:].rearrange("p (c g) -> p c g", g=3)
                gp.tensor_tensor(DAv, V[0], Xv, op=OP.subtract)
                DB = pool.tile([P, 3 * C], F32, name="DB")
                DBv = DB[:].rearrange("p (c g) -> p c g", g=3)
                gp.tensor_tensor(DBv, V[2], Yv, op=OP.subtract)

                A = pool.tile([P, 3 * C], BF16, name="A")
                dve.tensor_scalar(A[:], DA[:], 0.0, None, op0=OP.is_gt)
                Bt = pool.tile([P, 3 * C], BF16, name="Bt")
                dve.tensor_scalar(Bt[:], DB[:], 0.0, None, op0=OP.is_gt)
                M = pool.tile([P, 3 * C], BF16, name="M")
                dve.tensor_tensor(M[:], A[:], Bt[:], op=OP.subtract)
                Mv = M[:].rearrange("p (c g) -> p c g", g=3)
                mu, mn, md = Mv[:, :, 0], Mv[:, :, 1], Mv[:, :, 2]

                # --- calc = |mn| * (mu + md + mn) ----------------------
                S1 = pool.tile([P, C], BF16, name="S1")
                dve.tensor_tensor(S1[:], mu, md, op=OP.add)
                S2 = pool.tile([P, C], BF16, name="S2")
                dve.tensor_tensor(S2[:], S1[:], mn, op=OP.add)
                T1 = pool.tile([P, C], BF16, name="T1")
                dve.tensor_tensor(T1[:], mn, S2[:], op=OP.mult)
                CALC = pool.tile([P, C], BF16, name="CALC")
                dve.tensor_tensor(CALC[:], mn, T1[:], op=OP.mult)

                SGN = pool.tile([P, C], BF16, name="SGN")
                act.sign(SGN[:], CALC[:])
                E0 = pool.tile([P, C], U8, name="E0")
                dve.tensor_scalar(E0[:], CALC[:], 1.0, None, op0=OP.is_equal)
                E1 = pool.tile([P, C], U8, name="E1")
                dve.tensor_scalar(E1[:], CALC[:], 0.0, None, op0=OP.is_equal)

                # --- keep_g = (m_g == sgn) -----------------------------
                KD = pool.tile([P, 3 * C], BF16, name="KD")
                KDv = KD[:].rearrange("p (c g) -> p c g", g=3)
                sgnb = SGN[:].broadcast_to((P, C, 3))
                gp.tensor_tensor(KDv, Mv, sgnb, op=OP.subtract)
                KEEP = pool.tile([P, 3 * C], F32, name="KEEP")
                KEEPv = KEEP[:].rearrange("p (c g) -> p c g", g=3)
                dve.tensor_scalar(KEEP[:], KD[:], 0.0, None, op0=OP.is_equal)

                # --- probe ---------------------------------------------
                PRraw = pool.tile([P, 3 * C], F32, name="PRraw")
                PRrawv = PRraw[:].rearrange("p (c g) -> p c g", g=3)
                act.copy(PRrawv, V[2])
                e1b = E1[:].broadcast_to((P, C, 3))
                e0b = E0[:].broadcast_to((P, C, 3))
                _copy_pred(dve, PRrawv, e1b, V[1])
                _copy_pred(dve, PRrawv, e0b, V[0])
                PR = pool.tile([P, 3 * C], F32, name="PR")
                PRv = PR[:].rearrange("p (c g) -> p c g", g=3)
                gp.tensor_tensor(PR[:], PRraw[:], KEEP[:], op=OP.mult)

                # --- choice --------------------------------------------
                CN = pool.tile([P, C], U8, name="CN")
                dve.tensor_tensor(CN[:], PRv[:, :, 1], PRv[:, :, 2], op=OP.is_ge)
                MND = pool.tile([P, C], F32, name="MND")
                dve.tensor_tensor(MND[:], PRv[:, :, 1], PRv[:, :, 2], op=OP.max)
                CU = pool.tile([P, C], U8, name="CU")
                dve.tensor_tensor(CU[:], PRv[:, :, 0], MND[:], op=OP.is_ge)
                cnb = CN[:].broadcast_to((P, C, 3))
                cub = CU[:].broadcast_to((P, C, 3))

                # --- output --------------------------------------------
                OTraw = pool.tile([P, 3 * C], F32, name="OTraw")
                OTrawv = OTraw[:].rearrange("p (c j) -> p c j", j=3)
                act.copy(OTrawv, G[2])
                _copy_pred(dve, OTrawv, cnb, G[1])
                _copy_pred(dve, OTrawv, cub, G[0])

                KSEL = pool.tile([P, C], F32, name="KSEL")
                act.copy(KSEL[:], KEEPv[:, :, 2])
                _copy_pred(dve, KSEL[:], CN[:], KEEPv[:, :, 1])
                _copy_pred(dve, KSEL[:], CU[:], KEEPv[:, :, 0])

                OT = pool.tile([P, 3 * C], F32, name="OT")
                OTv = OT[:].rearrange("p (c j) -> p c j", j=3)
                kselb = KSEL[:].broadcast_to((P, C, 3))
                gp.tensor_tensor(OTv, OTrawv, kselb, op=OP.mult)

                nc.sync.dma_start(ot[t], OT[:])

    return nc


_BUILT = {}


def _get_nc(R: int, C: int):
    key = (R, C)
    if key not in _BUILT:
        nc = build_nc(R, C, bufs=int(os.environ.get("KERNEL_BUFS", "2")))
        split_multi_waits(nc)
        _BUILT[key] = nc
    return _BUILT[key]


KERNEL_C = int(os.environ.get("KERNEL_C", "512"))


def kernel(inputs) -> np.ndarray:
    x = np.ascontiguousarray(np.asarray(inputs, dtype=np.float32))
    n = x.shape[0]
    assert n % N_CORES == 0
    r = n // N_CORES
    nc = _get_nc(r, KERNEL_C)
    shards = x.reshape(N_CORES, r, 9)
    from concourse.bass_utils import run_bass_kernel_spmd

    res = run_bass_kernel_spmd(
        nc, [{"x": shards[i]} for i in range(N_CORES)], core_ids=list(range(N_CORES))
    )
    return np.concatenate([r_["o"] for r_ in res.results], axis=0)
